# revision 6
# baseline (speedup 1.0000x reference)
"""Trainium2 Bass kernel for nn_AttentionModel (sparse banded attention).

Math (per batch element, data-parallel over 8 cores):
  qs    = q @ W_score.T
  score = qs @ k.T                      # only the 129-wide causal band matters
  w     = banded_softmax(score)         # full-row max cancels mathematically
  c     = w @ k
  enh   = tanh(concat([c, q]) @ W_enh.T + b_enh)
  out   = sigmoid(enh @ W_mask.T + b_mask)

Implementation notes (v1 rewrite):
  - All matmul operands are bf16 (host-cast); PSUM accumulation stays fp32.
    Simulated end-to-end rel-err of the bf16 pipeline is ~9.6e-3 (tol 2e-2).
  - T=2000 padded: keys 128 zero rows front + 48 tail -> 2176 = 17*128;
    queries 48 tail -> 2048 = 16*128.  Query tile j attends padded key
    blocks j (prev) and j+1 (diag).
  - Weights/consts packed into two bf16 DRAM blobs + one tiny fp32 blob so
    the prologue is 3 big DMAs instead of ~20 small ones; k/q are loaded as
    a few large fully-contiguous transfers (kN is pre-shuffled to p-major
    layout on the host so its DMA needs no gather).
  - Scores for a PAIR of query tiles share one PSUM bank [128,512]; the band
    mask is ONE accumulated identity-matmul per pair; exp runs per tile with
    accum_out giving row sums for free.
  - Transposed softmax weights land in PSUM as [t0p|t0d|t1p|t1d] so PV needs
    no zero-half padding: per h, 3 matmuls (N=128/256/128) using per-element
    PSUM has_written accumulation.  One contiguous DVE copy evacuates them.
  - sigmoid(x) = 0.5*tanh(0.5x)+0.5: W_mask/b_mask are pre-scaled by 0.5 on
    the host, the kernel emits tanh values, and the final 0.5*x+0.5 affine
    plus column-256 assembly happen on the host (free).
  - The 257th output column is computed by N=1 matmuls into a persistent
    [128,16] PSUM bank, activated once at the end.
"""

import sys
import types

import numpy as np
import ml_dtypes
from contextlib import ExitStack

import concourse.bass as bass
import concourse.bacc as bacc
import concourse.tile as tile
from concourse import mybir
from concourse.bass_utils import run_bass_kernel_spmd


def _ensure_axon_hooks():
    try:
        from antenv import axon_hooks  # noqa: F401
        return
    except ImportError:
        pass
    try:
        from trn_agent_boot.trn_boot import _ntff_profile_via_ctypes
        hook = _ntff_profile_via_ctypes("/opt/axon/libaxon_pjrt.so")
    except Exception:
        hook = None
    m = types.ModuleType("antenv.axon_hooks")
    m.get_axon_ntff_profile_hook = lambda: hook
    m.set_axon_ntff_profile_hook = lambda h: None
    sys.modules["antenv.axon_hooks"] = m


_ensure_axon_hooks()

F32 = mybir.dt.float32
BF16 = mybir.dt.bfloat16
AF = mybir.ActivationFunctionType

B, T, H, F_OUT = 8, 2000, 256, 257
TPK = 2176   # padded key length   (128 front + 2000 + 48 tail)
TPQ = 2048   # padded query length (2000 + 48 tail)
NT = 16      # query tiles of 128
NKB = 17     # key blocks of 128
NEG = -32768.0
N_CORES = 8
NPBF = np.dtype(ml_dtypes.bfloat16)

# packA column offsets (bf16 [128, 1664])
PA_WST = 0        # wst_h0 [128,256], wst_h1 [128,256]
PA_ID = 512       # identity [128,128]
PA_M0 = 640       # mask for pair 0 [128,512]
PA_MS = 1152      # mask standard pair [128,512]
PA_N = 1664

# packB column offsets (bf16 [128, 2178])
PB_WET = 0        # 4x [128,256]
PB_WMT = 1024     # 2x [128,256]
PB_WML = 1536     # 2x [128,1]
PB_ONE = 1538     # ones row [1,128] (row 0)
PB_BM = 1666      # bias row [1,512] (row 0)
PB_N = 2178

_CACHE = {}


def _pack_consts(W_score, W_enh, b_enh, W_mask, b_mask):
    t_i = np.arange(128, dtype=np.int32)[:, None]
    s_i = np.arange(128, dtype=np.int32)[None, :]
    m_prev = np.where(s_i >= t_i, 0.0, NEG).astype(np.float32)
    m_diag = np.where(s_i <= t_i, 0.0, NEG).astype(np.float32)
    m_full = np.full((128, 128), NEG, np.float32)

    packA = np.zeros((128, PA_N), np.float32)
    WsT = W_score.T.astype(np.float32)                      # [h, g]
    packA[:, 0:256] = WsT[0:128]
    packA[:, 256:512] = WsT[128:256]
    packA[:, PA_ID:PA_ID + 128] = np.eye(128, dtype=np.float32)
    packA[:, PA_M0:PA_M0 + 512] = np.concatenate(
        [m_full, m_diag, m_prev, m_diag], axis=1)
    packA[:, PA_MS:PA_MS + 512] = np.concatenate(
        [m_prev, m_diag, m_prev, m_diag], axis=1)

    packB = np.zeros((128, PB_N), np.float32)
    WeT = W_enh.T.astype(np.float32)                        # [d, f]
    for d in range(4):
        packB[:, d * 256:(d + 1) * 256] = WeT[d * 128:(d + 1) * 128]
    WmT = 0.5 * W_mask.T.astype(np.float32)                 # [f, o], pre-scaled
    packB[:, PB_WMT:PB_WMT + 256] = WmT[0:128, 0:256]
    packB[:, PB_WMT + 256:PB_WMT + 512] = WmT[128:256, 0:256]
    packB[:, PB_WML:PB_WML + 1] = WmT[0:128, 256:257]
    packB[:, PB_WML + 1:PB_WML + 2] = WmT[128:256, 256:257]
    packB[0, PB_ONE:PB_ONE + 128] = 1.0
    bm = 0.5 * b_mask.astype(np.float32)
    packB[0, PB_BM:PB_BM + 256] = bm[0:256]
    packB[0, PB_BM + 256:PB_BM + 512] = bm[0:256]

    pack32 = np.zeros((128, 4), np.float32)
    pack32[:, 0] = b_enh[0:128]
    pack32[:, 1] = b_enh[128:256]
    pack32[:, 2] = bm[256]

    return (packA.astype(NPBF), packB.astype(NPBF), pack32)


def build_nc():
    nc = bacc.Bacc("TRN2", target_bir_lowering=False, debug=False,
                   num_devices=N_CORES)

    kT = nc.declare_dram_parameter("kT", [H, TPK], BF16, isOutput=False)
    kN = nc.declare_dram_parameter("kN", [128, NKB * 256], BF16, isOutput=False)
    qT = nc.declare_dram_parameter("qT", [H, TPQ], BF16, isOutput=False)
    pA = nc.declare_dram_parameter("pA", [128, PA_N], BF16, isOutput=False)
    pB = nc.declare_dram_parameter("pB", [128, PB_N], BF16, isOutput=False)
    p32 = nc.declare_dram_parameter("p32", [128, 4], F32, isOutput=False)
    out_m = nc.declare_dram_parameter("out_m", [512, 1024], F32, isOutput=True)
    out_l = nc.declare_dram_parameter("out_l", [128, 16], F32, isOutput=True)

    with tile.TileContext(nc) as tc, ExitStack() as ctx:
        const = ctx.enter_context(tc.tile_pool(name="const", bufs=1))
        io = ctx.enter_context(tc.tile_pool(name="io", bufs=1))
        wk = ctx.enter_context(tc.tile_pool(name="wk", bufs=4))
        stat = ctx.enter_context(tc.tile_pool(name="stat", bufs=8))
        ob = ctx.enter_context(tc.tile_pool(name="ob", bufs=2))
        pmm = ctx.enter_context(tc.tile_pool(name="pmm", bufs=2, space="PSUM"))
        psc = ctx.enter_context(tc.tile_pool(name="psc", bufs=2, space="PSUM"))
        pwt = ctx.enter_context(tc.tile_pool(name="pwt", bufs=2, space="PSUM"))
        pct = ctx.enter_context(tc.tile_pool(name="pct", bufs=1, space="PSUM"))
        plz = ctx.enter_context(tc.tile_pool(name="plz", bufs=1, space="PSUM"))

        # ---- SBUF persistent tensors ----
        # Three DMA queues in parallel: scalar (HWDGE): packA + kT + pack32;
        # sync (HWDGE): qT + output stores; gpsimd (SWDGE): kN + packB.
        packA = const.tile([128, PA_N], BF16, tag="pA", name="packA")
        nc.scalar.dma_start(packA[:], pA[:])
        qT_t = [io.tile([128, TPQ], BF16, tag=f"qT{c}", name=f"qT{c}")
                for c in range(2)]
        kT_t = [io.tile([128, TPK], BF16, tag=f"kT{c}", name=f"kT{c}")
                for c in range(2)]
        kN_t = io.tile([128, NKB * 256], BF16, tag="kN", name="kN_t")
        packB = const.tile([128, PB_N], BF16, tag="pB", name="packB")
        pack32 = const.tile([128, 4], F32, tag="p32", name="pack32")

        for half in range(2):
            for c in range(2):
                nc.sync.dma_start(
                    qT_t[c][:, half * 1024:(half + 1) * 1024],
                    qT[c * 128:(c + 1) * 128, half * 1024:(half + 1) * 1024])
        for half in range(2):
            for c in range(2):
                nc.scalar.dma_start(
                    kT_t[c][:, half * 1088:(half + 1) * 1088],
                    kT[c * 128:(c + 1) * 128, half * 1088:(half + 1) * 1088])
        nc.scalar.dma_start(pack32[:], p32[:])
        kn_groups = [(0, 5), (5, 9), (9, 13), (13, 17)]
        for i, (b0, b1) in enumerate(kn_groups):
            nc.gpsimd.dma_start(kN_t[:, b0 * 256:b1 * 256],
                                kN[:, b0 * 256:b1 * 256])
            if i == 0:
                nc.gpsimd.dma_start(packB[:], pB[:])

        qsT_t = [io.tile([128, TPQ], BF16, tag=f"qsT{c}", name=f"qsT{c}")
                 for c in range(2)]
        cT_t = io.tile([128, 2 * TPQ], BF16, tag="cT", name="cT_t")
        enhT_t = io.tile([128, 2 * TPQ], BF16, tag="enhT", name="enhT_t")
        lastz = plz.tile([128, 16], F32, tag="lz", name="lastz")

        ident = packA[:, PA_ID:PA_ID + 128]

        # ---- P0: qsT[g, t'] = (q @ W_score.T).T ----
        for nb in range(4):         # 512-wide t' blocks (nb-major: unblocks
            for c in range(2):      # P1 pair 0 as early as possible)
                ps = pmm.tile([128, 512], F32, tag="mm", name="ps")
                for h in range(2):  # contraction chunk
                    nc.tensor.matmul(
                        ps[:],
                        packA[:, h * 256 + c * 128: h * 256 + (c + 1) * 128],
                        qT_t[h][:, nb * 512:(nb + 1) * 512],
                        start=(h == 0), stop=(h == 1))
                nc.vector.tensor_copy(qsT_t[c][:, nb * 512:(nb + 1) * 512],
                                      ps[:])

        # ---- P1 per pair: scores -> softmax -> transposed weights -> PV ----
        def p1(p):
            j0, j1 = 2 * p, 2 * p + 1
            sc = psc.tile([128, 512], F32, tag="sc", name="sc")
            for l, j in ((0, j0), (1, j1)):
                for c in range(2):
                    nc.tensor.matmul(
                        sc[:, l * 256:(l + 1) * 256],
                        qsT_t[c][:, j * 128:(j + 1) * 128],
                        kT_t[c][:, j * 128: j * 128 + 256],
                        start=(l == 0 and c == 0), stop=False,
                        skip_group_check=True)
            moff = PA_M0 if p == 0 else PA_MS
            nc.tensor.matmul(sc[:], ident, packA[:, moff:moff + 512],
                             start=False, stop=True, skip_group_check=True)
            # exp with per-tile row sums; normalize; transpose into PV layout
            e_t = wk.tile([128, 512], BF16, tag="e", name="e_t")
            den = stat.tile([128, 2], F32, tag="den", name="den")
            for l in range(2):
                nc.scalar.activation(e_t[:, l * 256:(l + 1) * 256],
                                     sc[:, l * 256:(l + 1) * 256],
                                     AF.Exp, accum_out=den[:, l:l + 1])
            rec = stat.tile([128, 2], F32, tag="rec", name="rec")
            nc.vector.reciprocal(rec[:], den[:])
            w_t = wk.tile([128, 512], BF16, tag="w", name="w_t")
            for l in range(2):
                nc.vector.tensor_scalar_mul(w_t[:, l * 256:(l + 1) * 256],
                                            e_t[:, l * 256:(l + 1) * 256],
                                            rec[:, l:l + 1])
            # [t0p|t0d|t1p|t1d] -> transposed halves, same order
            pw = pwt.tile([128, 512], BF16, tag="pw", name="pw")
            for r in range(4):
                nc.tensor.transpose(pw[:, r * 128:(r + 1) * 128],
                                    w_t[:, r * 128:(r + 1) * 128], ident)
            wTP = wk.tile([128, 512], BF16, tag="wTP", name="wTP")
            nc.vector.tensor_copy(wTP[:], pw[:])
            # PV: c.T[h, t'pair] over 3 key blocks (2p, 2p+1, 2p+2)
            pc = pct.tile([128, 512], F32, tag="pc", name="pc")
            first = True
            for h in range(2):
                base = h * 256
                for m, (lo, hi) in ((0, (0, 128)), (1, (128, 384)),
                                    (2, (384, 512))):
                    blk = 2 * p + m
                    dst_lo = base + (0 if m == 0 else (0 if m == 1 else 128))
                    dst_hi = base + (128 if m == 0 else (256 if m == 1 else 256))
                    nc.tensor.matmul(
                        pc[:, dst_lo:dst_hi],
                        kN_t[:, blk * 256 + h * 128: blk * 256 + (h + 1) * 128],
                        wTP[:, lo:hi],
                        start=first, stop=(h == 1 and m == 2),
                        skip_group_check=True)
                    first = False
            dst = cT_t[:].rearrange("p (h t) -> p h t", h=2)[
                :, :, 2 * p * 128: 2 * p * 128 + 256]
            src = pc[:].rearrange("p (h t) -> p h t", h=2)
            nc.vector.tensor_copy(dst, src)

        # ---- P2 per nb: enhT[f, t'] = tanh(W_enh.T @ [cT;qT] + b_enh) ----
        def p2(nb):
            for f in range(2):
                pe_ = pmm.tile([128, 512], F32, tag="mm", name="pe_")
                for d in range(4):
                    if d < 2:
                        rhs = cT_t[:, d * TPQ + nb * 512: d * TPQ + (nb + 1) * 512]
                    else:
                        rhs = qT_t[d - 2][:, nb * 512:(nb + 1) * 512]
                    nc.tensor.matmul(
                        pe_[:],
                        packB[:, d * 256 + f * 128: d * 256 + (f + 1) * 128],
                        rhs, start=(d == 0), stop=(d == 3))
                nc.scalar.activation(
                    enhT_t[:, f * TPQ + nb * 512: f * TPQ + (nb + 1) * 512],
                    pe_[:], AF.Tanh, bias=pack32[:, f:f + 1])

        # ---- P3 per pair: z' = 0.5*(enh @ W_mask.T + b_mask); emit tanh(z')
        def p3(pp, obuf):
            pm = pmm.tile([128, 512], F32, tag="mm", name="pm")
            for l, j in ((0, 2 * pp), (1, 2 * pp + 1)):
                for f in range(2):
                    enh_sl = enhT_t[:, f * TPQ + j * 128: f * TPQ + (j + 1) * 128]
                    nc.tensor.matmul(
                        pm[:, l * 256:(l + 1) * 256],
                        enh_sl, packB[:, PB_WMT + f * 256: PB_WMT + (f + 1) * 256],
                        start=(l == 0 and f == 0), stop=False,
                        skip_group_check=True)
                    nc.tensor.matmul(
                        lastz[:, j:j + 1],
                        enh_sl, packB[:, PB_WML + f: PB_WML + f + 1],
                        start=(j == 0 and f == 0), stop=(f == 1),
                        skip_group_check=True)
            nc.tensor.matmul(pm[:], packB[0:1, PB_ONE:PB_ONE + 128],
                             packB[0:1, PB_BM:PB_BM + 512],
                             start=False, stop=True, skip_group_check=True)
            nc.scalar.activation(obuf[:, (pp % 2) * 512:(pp % 2) * 512 + 512],
                                 pm[:], AF.Tanh)

        for p in range(NT // 2):
            p1(p)
            if p % 2 == 1:
                nb = p // 2
                p2(nb)
                obuf = ob.tile([128, 1024], F32, tag="ob", name="obuf")
                p3(p - 1, obuf)
                p3(p, obuf)
                nc.sync.dma_start(out_m[nb * 128:(nb + 1) * 128, :], obuf[:])

        ol = stat.tile([128, 16], F32, tag="ol", name="ol")
        nc.scalar.activation(ol[:], lastz[:], AF.Tanh, bias=pack32[:, 2:3])
        nc.sync.dma_start(out_l[:], ol[:])

    return nc


def make_in_maps(k, q, W_score, W_enh, b_enh, W_mask, b_mask):
    packA, packB, pack32 = _pack_consts(
        np.asarray(W_score, np.float32), np.asarray(W_enh, np.float32),
        np.asarray(b_enh, np.float32), np.asarray(W_mask, np.float32),
        np.asarray(b_mask, np.float32))
    k = np.asarray(k, np.float32)
    q = np.asarray(q, np.float32)
    in_maps = []
    for b in range(N_CORES):
        kpad = np.zeros((TPK, H), np.float32)
        kpad[128:128 + T] = k[b]
        kb = kpad.astype(NPBF)
        qpad = np.zeros((TPQ, H), np.float32)
        qpad[:T] = q[b]
        qb = qpad.astype(NPBF)
        # p-major shuffle for kN: kN[p, blk*256+h] = kpad[blk*128+p, h]
        kNh = np.ascontiguousarray(
            kb.reshape(NKB, 128, H).transpose(1, 0, 2).reshape(128, NKB * 256))
        in_maps.append({
            "kT": np.ascontiguousarray(kb.T),
            "kN": kNh,
            "qT": np.ascontiguousarray(qb.T),
            "pA": packA, "pB": packB, "p32": pack32,
        })
    return in_maps


def assemble(results):
    outs = []
    for r in results:
        main = r["out_m"].reshape(4, 128, 4, 256).transpose(0, 2, 1, 3)
        main = main.reshape(TPQ, 256)
        last = np.ascontiguousarray(r["out_l"].T).reshape(TPQ)
        full = np.empty((TPQ, F_OUT), np.float32)
        full[:, :256] = 0.5 * main + 0.5
        full[:, 256] = 0.5 * last + 0.5
        outs.append(full[:T])
    return np.stack(outs, 0)


def get_nc():
    if "nc" not in _CACHE:
        nc = build_nc()
        nc.finalize()
        _CACHE["nc"] = nc
    return _CACHE["nc"]


def kernel(k, q, W_score, W_enh, b_enh, W_mask, b_mask):
    in_maps = make_in_maps(k, q, W_score, W_enh, b_enh, W_mask, b_mask)
    res = run_bass_kernel_spmd(get_nc(), in_maps, list(range(N_CORES)))
    return assemble(res.results)
